# revision 36
# baseline (speedup 1.0000x reference)
"""BiDAF attention + masked max-pool + classifier kernel for Trainium2.

Reference computation (per batch b):
  S = H @ W_attn @ U^T                       (P, Q)
  c2q = softmax_q(S) @ U                     (P, D)
  b_attn = softmax_p(max_q S)                (P,)
  q2c = b_attn @ H                           (D,)
  G_M = [H; c2q; H*c2q; H*q2c; M]            (P, 5D)
  pooled = max over non-pad p of G_M         (5D,)
  out = pooled @ W_cls                       (2,)

Sharding: data-parallel over batch. B=32 -> 8 cores x 4 batches.

Design notes:
  * Host uploads bf16 copies of H (both layouts: H^T [d,p] and natural
    [l,c,d]) and M^T [d,p].  Pad rows (sentence_word_rep==0) are replaced
    host-side with a copy of the first non-pad row: the pooled maxes are
    then plain (unmasked) maxes; the b_attn/q2c perturbation is O(weight
    of one random row in a 4096-row softmax) ~ 1e-7 (verified on the
    actual inputs in numcheck.py).
  * b_attn = softmax_p(max_q S) is computed as maxE/sum(maxE) with
    maxE = max_q exp(S): exp is monotone, |S| <= ~70 so exp(S) is in
    fp32/bf16 range, and no log/exp epilogue is needed.
  * softmax_q skips max subtraction; Z = sum_q exp(S) per row.
  * probs are transposed [p,q] -> [q,p] by the DMA XBAR transpose
    (2-byte dtype): out[:, t, :] = in[:, 128t:128(t+1)].T, so the two
    64-row chunk transposes land stacked in one 128-partition tile.
  * Pooling streams (H^T, c2q^T, H^T*c2q^T) are bf16 [d, p] tiles folded
    with tensor_tensor max/min (2x DVE mode) into [d, 1024] accs,
    halved on Pool and reduced on DVE once per batch.
  * M is max-accumulated by the DMA itself (gpsimd software DGE with
    accum_op=max, dst access pattern revisiting the same [128,1024]
    region) - the M stream never touches a compute engine.
  * Two batches are interleaved block-by-block to keep every in-order
    engine queue supplied with independent work.
"""

import sys

for _p in ("/opt/trn_rl_repo", "/opt/trn_rl_repo/concourse"):
    if _p not in sys.path:
        sys.path.insert(0, _p)

from contextlib import ExitStack

import numpy as np

import concourse.bass as bass
import concourse.tile as tile
from concourse import bacc, bass_isa, masks, mybir
from concourse.bass_utils import run_bass_kernel_spmd

F32 = mybir.dt.float32
BF16 = mybir.dt.bfloat16
ALU = mybir.AluOpType
AF = mybir.ActivationFunctionType

N_CORES = 8
B, P, Q, D = 32, 4096, 64, 128
B_CORE = B // N_CORES          # 4 batches per core
NBLK = 4                       # 1024-p blocks per batch
BLK = P // NBLK                # 1024
NCH = BLK // 128               # 8 chunks of 128 p per block

USE_M_DMA_ACCUM = False


def build_program():
    nc = bacc.Bacc("TRN2", target_bir_lowering=False, debug=False,
                   num_devices=N_CORES)

    ht_ext = nc.dram_tensor("ht16", [B_CORE, D, P], BF16, kind="ExternalInput").ap()
    hn_ext = nc.dram_tensor("hn16", [B_CORE, 128, P // 128, D], BF16,
                            kind="ExternalInput").ap()
    mt_ext = nc.dram_tensor("mt16", [B_CORE, D, P], BF16, kind="ExternalInput").ap()
    u_ext = nc.dram_tensor("u16", [B_CORE, 2 * Q, D], BF16, kind="ExternalInput").ap()
    ut_ext = nc.dram_tensor("ut32", [B_CORE, D, Q], F32, kind="ExternalInput").ap()
    wt_ext = nc.dram_tensor("wt32", [D, D], F32, kind="ExternalInput").ap()
    wcls_ext = nc.dram_tensor("wcls", [5 * D, 2], F32, kind="ExternalInput").ap()
    out_ext = nc.dram_tensor("out", [B_CORE, 2], F32, kind="ExternalOutput").ap()

    with tile.TileContext(nc) as tc, ExitStack() as ctx:
        pool1 = ctx.enter_context(tc.tile_pool(name="const", bufs=1))
        poolb = ctx.enter_context(tc.tile_pool(name="batch", bufs=3))
        poolk = ctx.enter_context(tc.tile_pool(name="blk", bufs=5))
        poolp = ctx.enter_context(tc.tile_pool(name="pairp", bufs=5))
        psS = ctx.enter_context(tc.tile_pool(name="psS", bufs=3, space="PSUM"))
        psCQ = ctx.enter_context(tc.tile_pool(name="psCQ", bufs=3, space="PSUM"))
        psW = ctx.enter_context(tc.tile_pool(name="psW", bufs=1, space="PSUM"))
        psE = ctx.enter_context(tc.tile_pool(name="psE", bufs=1, space="PSUM"))

        ident16 = pool1.tile([128, 128], BF16)
        masks.make_identity(nc, ident16[:])
        ident32 = pool1.tile([128, 128], F32)
        masks.make_identity(nc, ident32[:])
        wt_sb = pool1.tile([D, D], F32)
        nc.sync.dma_start(wt_sb[:], wt_ext[:])
        wcls_sb = pool1.tile([D, 5, 2], F32)
        nc.sync.dma_start(wcls_sb[:], wcls_ext.rearrange("(k d) o -> d k o", k=5))

        def emit_prologue(b, st, part):
            if part == 0:
                # critical loads first: ht (S input), ut (Wu), u
                ht16 = poolb.tile([D, P], BF16, tag="ht")
                nc.sync.dma_start(ht16[:], ht_ext[b])
                ut32 = poolb.tile([D, Q], F32, tag="ut")
                nc.sync.dma_start(ut32[:], ut_ext[b])
                u16 = poolb.tile([2 * Q, D], BF16, tag="u")
                nc.sync.dma_start(u16[:], u_ext[b])

                wu_ps = psW.tile([D, Q], F32, tag="wu")
                nc.tensor.matmul(wu_ps[:], lhsT=wt_sb[:], rhs=ut32[:],
                                 start=True, stop=True)
                wu16 = poolb.tile([D, Q], BF16, tag="wu16")
                nc.scalar.copy(wu16[:], wu_ps[:])

                maxE16 = poolb.tile([128, P // 128], BF16, tag="maxE")
                accCP = poolb.tile([D, 2, BLK], BF16, tag="accCP")
                accH = poolb.tile([D, BLK], BF16, tag="accH")
                accHm = poolb.tile([D, BLK], BF16, tag="accHm")
                st.update(ht16=ht16, u16=u16, wu16=wu16, maxE16=maxE16,
                          accCP=accCP, accH=accH, accHm=accHm, pend=None)
                return

            # non-critical loads: hn (epilogue only), M (epilogue only)
            hn16 = poolb.tile([128, P // 128, D], BF16, tag="hn")
            nc.sync.dma_start(hn16[:], hn_ext[b])
            mt16 = poolb.tile([D, P], BF16, tag="mt")
            nc.sync.dma_start(mt16[:], mt_ext[b])
            st.update(hn16=hn16, mt16=mt16)

        def emit_block(b, st, kb):
            """Stage A of block kb (S/exp/softmax/XBAR/c2q matmuls) plus
            stage B of block kb-1 (c2q copy, prod, folds)."""
            ht16 = st["ht16"]
            u16 = st["u16"]
            wu16 = st["wu16"]
            if kb < NBLK:
                p0 = kb * BLK
                s_ps = psS.tile([128, NCH, Q], F32, tag="s")
                for c in range(NCH):
                    nc.tensor.matmul(
                        s_ps[:, c, :],
                        lhsT=ht16[:, p0 + c * 128:p0 + (c + 1) * 128],
                        rhs=wu16[:], start=(c == 0), stop=(c == NCH - 1),
                        skip_group_check=True)

                probs16 = poolk.tile([128, NCH, Q], BF16, tag="probs")
                nc.scalar.activation(probs16[:], s_ps[:], AF.Exp)

                zc = poolk.tile([128, NCH], F32, tag="zc")
                nc.vector.reduce_sum(zc[:], probs16[:], axis=mybir.AxisListType.X)
                nc.vector.reduce_max(st["maxE16"][:, kb * NCH:(kb + 1) * NCH],
                                     probs16[:], axis=mybir.AxisListType.X)

                rz = poolk.tile([128, NCH], F32, tag="rz")
                nc.vector.reciprocal(rz[:], zc[:])
                nc.gpsimd.tensor_tensor(
                    out=probs16[:], in0=probs16[:],
                    in1=rz[:, :, None].broadcast_to((128, NCH, Q)), op=ALU.mult)

                ptT = psS.tile([Q, NCH, 128], BF16, tag="s")
                for c in range(NCH):
                    nc.tensor.matmul(ptT[:, c, :], lhsT=probs16[:, c, :],
                                     rhs=ident16[:], is_transpose=True,
                                     start=(c == 0), stop=(c == NCH - 1),
                                     skip_group_check=True)
                pt16 = poolk.tile([Q, NCH, 128], BF16, tag="pt")
                nc.scalar.copy(pt16[:], ptT[:])

                halves = []
                for h in range(2):
                    c2q_ps = psCQ.tile([D, BLK // 2], F32, tag="c2q")
                    for cc in range(NCH // 2):
                        c = h * (NCH // 2) + cc
                        nc.tensor.matmul(
                            c2q_ps[:, cc * 128:(cc + 1) * 128],
                            lhsT=u16[0:Q, :],
                            rhs=pt16[:, c, :],
                            start=(cc == 0), stop=(cc == NCH // 2 - 1),
                            skip_group_check=True)
                    halves.append(c2q_ps)
                cur = (halves, p0)
            else:
                cur = None

            if st["pend"] is not None:
                halves, q0 = st["pend"]
                accCP, accH, accHm = st["accCP"], st["accH"], st["accHm"]
                pair16 = poolp.tile([D, 2, BLK], BF16, tag="pair")
                for h in range(2):
                    nc.scalar.copy(
                        pair16[:, 0, h * (BLK // 2):(h + 1) * (BLK // 2)],
                        halves[h][:])
                nc.vector.tensor_tensor(out=pair16[:, 1, :],
                                        in0=ht16[:, q0:q0 + BLK],
                                        in1=pair16[:, 0, :], op=ALU.mult)
                if q0 == 0:
                    nc.vector.tensor_copy(accCP[:], pair16[:])
                    nc.vector.tensor_copy(accH[:], ht16[:, q0:q0 + BLK])
                    nc.vector.tensor_copy(accHm[:], ht16[:, q0:q0 + BLK])
                else:
                    nc.vector.tensor_tensor(out=accCP[:], in0=accCP[:],
                                            in1=pair16[:], op=ALU.max)
                    nc.vector.tensor_tensor(out=accH[:], in0=accH[:],
                                            in1=ht16[:, q0:q0 + BLK],
                                            op=ALU.max)
                    nc.vector.tensor_tensor(out=accHm[:], in0=accHm[:],
                                            in1=ht16[:, q0:q0 + BLK],
                                            op=ALU.min)
            st["pend"] = cur

        def emit_epilogue(b, st):
            hn16, mt16 = st["hn16"], st["mt16"]
            maxE16 = st["maxE16"]
            accCP, accH, accHm = st["accCP"], st["accH"], st["accHm"]

            # first fold step on Pool (halve widths), final reduces on DVE
            e0CP = poolb.tile([D, 2, BLK], BF16, tag="e0CP")
            nc.gpsimd.tensor_tensor(out=e0CP[:], in0=accCP[:, :, :BLK],
                                    in1=accCP[:, :, BLK:], op=ALU.max)
            e1CP = poolb.tile([D, 2, BLK // 2], BF16, tag="e1CP")
            nc.gpsimd.tensor_tensor(out=e1CP[:], in0=e0CP[:, :, :BLK // 2],
                                    in1=e0CP[:, :, BLK // 2:], op=ALU.max)
            e1H = poolb.tile([D, BLK // 2], BF16, tag="e1H")
            nc.gpsimd.tensor_tensor(out=e1H[:], in0=accH[:, :BLK // 2],
                                    in1=accH[:, BLK // 2:], op=ALU.max)
            e1Hm = poolb.tile([D, BLK // 2], BF16, tag="e1Hm")
            nc.gpsimd.tensor_tensor(out=e1Hm[:], in0=accHm[:, :BLK // 2],
                                    in1=accHm[:, BLK // 2:], op=ALU.min)
            e0M = poolb.tile([D, BLK], BF16, tag="e0M")
            nc.gpsimd.tensor_tensor(out=e0M[:], in0=accM[:, :BLK],
                                    in1=accM[:, BLK:], op=ALU.max)
            e1M = poolb.tile([D, BLK // 2], BF16, tag="e1M")
            nc.gpsimd.tensor_tensor(out=e1M[:], in0=e0M[:, :BLK // 2],
                                    in1=e0M[:, BLK // 2:], op=ALU.max)

            pooled = poolb.tile([D, 5], F32, tag="pooled")
            nc.vector.reduce_max(pooled[:, 1:3], e1CP[:],
                                 axis=mybir.AxisListType.X)
            nc.vector.reduce_max(pooled[:, 0, None], e1H[:],
                                 axis=mybir.AxisListType.X)
            minH = poolb.tile([D, 1], F32, tag="minH")
            nc.vector.tensor_reduce(minH[:], e1Hm[:],
                                    axis=mybir.AxisListType.X, op=ALU.min)
            nc.vector.reduce_max(pooled[:, 4, None], e1M[:],
                                 axis=mybir.AxisListType.X)

            # q2c (unnormalized): sum_p maxE_p * H[p, :]
            q2c_ps = psE.tile([D, 1], F32, tag="eps")
            for c in range(P // 128):
                nc.tensor.matmul(q2c_ps[:], lhsT=hn16[:, c, :],
                                 rhs=maxE16[:, c, None],
                                 start=(c == 0), stop=(c == P // 128 - 1))

            # Zb = sum_p maxE_p, broadcast to all partitions
            zbcol = poolb.tile([128, 1], F32, tag="zbcol")
            nc.vector.reduce_sum(zbcol[:], maxE16[:], axis=mybir.AxisListType.X)
            zball = poolb.tile([128, 1], F32, tag="zball")
            nc.gpsimd.partition_all_reduce(zball[:], zbcol[:], channels=128,
                                           reduce_op=bass_isa.ReduceOp.add)
            rzb = poolb.tile([128, 1], F32, tag="rzb")
            nc.vector.reciprocal(rzb[:], zball[:])

            # pooled[:,3] = max(q2cu*maxH, q2cu*minH) / Zb
            q2cu = poolb.tile([D, 1], F32, tag="q2cu")
            nc.vector.tensor_copy(q2cu[:], q2c_ps[:])
            t1 = poolb.tile([D, 1], F32, tag="t1")
            nc.vector.tensor_tensor(out=t1[:], in0=q2cu[:],
                                    in1=pooled[:, 0, None], op=ALU.mult)
            t2 = poolb.tile([D, 1], F32, tag="t2")
            nc.vector.tensor_tensor(out=t2[:], in0=q2cu[:], in1=minH[:],
                                    op=ALU.mult)
            nc.vector.tensor_tensor(out=t1[:], in0=t1[:], in1=t2[:], op=ALU.max)
            nc.vector.tensor_tensor(out=pooled[:, 3, None], in0=t1[:],
                                    in1=rzb[:], op=ALU.mult)

            # classifier: out[1,2] = sum_k pooled[:,k]^T @ Wcls[k]
            out_ps = psE.tile([1, 2], F32, tag="eps")
            for k in range(5):
                nc.tensor.matmul(out_ps[:], lhsT=pooled[:, k, None],
                                 rhs=wcls_sb[:, k, :],
                                 start=(k == 0), stop=(k == 4))
            out_sb = poolb.tile([1, 2], F32, tag="out_sb")
            nc.scalar.copy(out_sb[:], out_ps[:])
            nc.sync.dma_start(out_ext[b, None, :], out_sb[:])

        # interleave all four batches block-by-block
        sts = [{} for _ in range(B_CORE)]
        for b in range(B_CORE):
            emit_prologue(b, sts[b], 0)
        for b in range(B_CORE):
            emit_prologue(b, sts[b], 1)
        for kb in range(NBLK + 1):
            for b in range(B_CORE):
                emit_block(b, sts[b], kb)
        for b in range(B_CORE):
            emit_epilogue(b, sts[b])

    nc.compile()
    return nc


_CACHED_NC = None


def _get_program():
    global _CACHED_NC
    if _CACHED_NC is None:
        _CACHED_NC = build_program()
    return _CACHED_NC


def make_in_maps(tensor_H, tensor_U, M, sentence_word_rep, W_attn, W_cls):
    import ml_dtypes
    BF = ml_dtypes.bfloat16

    H = np.asarray(tensor_H, dtype=np.float32).copy()
    U = np.asarray(tensor_U, dtype=np.float32)
    Mm = np.asarray(M, dtype=np.float32).copy()
    W = np.asarray(W_attn, dtype=np.float32)
    Wc = np.ascontiguousarray(np.asarray(W_cls, dtype=np.float32))
    swr = np.asarray(sentence_word_rep)

    # replace pad rows with a copy of the first non-pad row (pooled maxes
    # become unmasked; b_attn/q2c perturbation ~1e-7, see numcheck)
    for b in range(B):
        pads = np.nonzero(swr[b] == 0)[0]
        if len(pads):
            nonpad = np.nonzero(swr[b] != 0)[0][0]
            H[b, pads] = H[b, nonpad]
            Mm[b, pads] = Mm[b, nonpad]

    ht16 = np.ascontiguousarray(H.transpose(0, 2, 1)).astype(BF)      # [B,D,P]
    hn16 = np.ascontiguousarray(
        H.reshape(B, P // 128, 128, D).transpose(0, 2, 1, 3)).astype(BF)
    mt16 = np.ascontiguousarray(Mm.transpose(0, 2, 1)).astype(BF)     # [B,D,P]
    u16 = np.concatenate([U, U], axis=1).astype(BF)                   # [B,2Q,D]
    ut32 = np.ascontiguousarray(U.transpose(0, 2, 1))                 # [B,D,Q]
    wt32 = np.ascontiguousarray(W.T)                                  # [e,d]

    in_maps = []
    for core in range(N_CORES):
        sl = slice(core * B_CORE, (core + 1) * B_CORE)
        in_maps.append({
            "ht16": np.ascontiguousarray(ht16[sl]),
            "hn16": np.ascontiguousarray(hn16[sl]),
            "mt16": np.ascontiguousarray(mt16[sl]),
            "u16": np.ascontiguousarray(u16[sl]),
            "ut32": np.ascontiguousarray(ut32[sl]),
            "wt32": wt32,
            "wcls": Wc,
        })
    return in_maps


def kernel(tensor_H, tensor_U, M, sentence_word_rep, W_attn, W_cls):
    nc = _get_program()
    in_maps = make_in_maps(tensor_H, tensor_U, M, sentence_word_rep,
                           W_attn, W_cls)
    res = run_bass_kernel_spmd(nc, in_maps, list(range(N_CORES)))
    out = np.concatenate([res.results[i]["out"] for i in range(N_CORES)], axis=0)
    return out.astype(np.float32)
